# revision 13
# baseline (speedup 1.0000x reference)
"""Localization-loss kernel for Trainium2, 8-core data-parallel SPMD.

Full inputs in, full (scalar) output out. Batch dim (128) is sharded 16
images per core; each core computes per-partition partial sums of the four
loss components plus per-row class-loss terms; the host does the final
~4KB reduction in float64.
"""

import numpy as np

_B, _H, _W, _NCLS = 128, 224, 224, 1000
_NCORES = 8
_BL = _B // _NCORES  # 16 images per core
_HW = _H * _W  # 50176
_P = 128  # SBUF partitions
_F = _HW // _P  # 392 elements per partition per image

_PROG = None  # cached Bass program (compile once per process)

# Max embedded sync waits per instruction struct for this compiler
# ("Too many sync wait commands" / "ISA wrong length"). Excess waits are
# hoisted into standalone EventSemaphore instructions just before them.
_MAX_EMBEDDED_WAITS = {"ISA": 0}
_DEFAULT_MAX_WAITS = 1


def _patch_multiwaits(bir_bytes):
    import json

    bir = json.loads(bir_bytes)
    for fn in bir["functions"]:
        for blk in fn["blocks"]:
            out = []
            for inst in blk["instructions"]:
                si = inst.get("sync_info")
                waits = (si or {}).get("on_wait") or []
                limit = _MAX_EMBEDDED_WAITS.get(
                    inst.get("opcode"), _DEFAULT_MAX_WAITS
                )
                if len(waits) > limit:
                    keep = waits[len(waits) - limit :] if limit else []
                    hoist = waits[: len(waits) - limit]
                    for k, w in enumerate(hoist):
                        out.append(
                            {
                                "debug": inst.get("debug", 0),
                                "engine": inst["engine"],
                                "ins": [],
                                "outs": [],
                                "name": f"{inst['name']}-hw{k}",
                                "opcode": "EventSemaphore",
                                "sync_info": {"on_update": [], "on_wait": [w]},
                            }
                        )
                    si["on_wait"] = keep
                out.append(inst)
            blk["instructions"] = out
    return json.dumps(bir).encode()

# Filled by the last _run() call; read by test.py for profiling info.
LAST_RESULTS = None


def _build():
    import concourse.bass as bass
    import concourse.tile as tile
    from concourse import mybir

    f32 = mybir.dt.float32
    Act = mybir.ActivationFunctionType
    Alu = mybir.AluOpType

    nc = bass.Bass()
    objects_h = nc.dram_tensor("objects", [_BL, _HW], f32, kind="ExternalInput")
    locs_h = nc.dram_tensor("locs", [_BL, 4, _HW], f32, kind="ExternalInput")
    gt_h = nc.dram_tensor("gt", [_BL, 5, _HW], f32, kind="ExternalInput")
    scores_h = nc.dram_tensor("scores", [_BL, _NCLS], f32, kind="ExternalInput")
    onehot_h = nc.dram_tensor("onehot", [_BL, _NCLS], f32, kind="ExternalInput")
    # out[:, 0] = per-partition coord-loss sums       (sum -> obj_coor_loss)
    # out[:, 1] = per-partition sum ln(1-p)           (A)
    # out[:, 2] = per-partition sum gt0*ln(1-p)       (B)
    # out[:, 3] = per-partition sum gt0*ln(p)         (C)
    # out[0:16, 4] = per-row class loss terms (max + lse - x[label])
    out_h = nc.dram_tensor("out", [_P, 5], f32, kind="ExternalOutput")

    with tile.TileContext(nc) as tc:
        with (
            tc.tile_pool(name="work", bufs=3) as work,
            tc.tile_pool(name="acc", bufs=1) as accp,
        ):
            acc_sq = accp.tile([_P, _BL], f32)
            acc_a = accp.tile([_P, _BL], f32)
            acc_b = accp.tile([_P, _BL], f32)
            acc_c = accp.tile([_P, _BL], f32)
            out_t = accp.tile([_P, 5], f32)
            nc.vector.memset(out_t, 0.0)

            # ---- image class loss: -(x[label] - rowmax - ln(sum exp(x-rowmax)))
            scores_t = accp.tile([_BL, _NCLS], f32)
            nc.sync.dma_start(out=scores_t, in_=scores_h[:])
            onehot_t = accp.tile([_BL, _NCLS], f32)
            nc.sync.dma_start(out=onehot_t, in_=onehot_h[:])
            rmax_t = accp.tile([_BL, 1], f32)
            nc.vector.reduce_max(out=rmax_t, in_=scores_t, axis=mybir.AxisListType.X)
            nmax_t = accp.tile([_BL, 1], f32)
            nc.vector.tensor_scalar_mul(nmax_t, rmax_t, -1.0)
            exp_t = accp.tile([_BL, _NCLS], f32)
            sumexp_t = accp.tile([_BL, 1], f32)
            nc.scalar.activation(
                out=exp_t,
                in_=scores_t,
                func=Act.Exp,
                bias=nmax_t,
                scale=1.0,
                accum_out=sumexp_t,
            )
            lse_t = accp.tile([_BL, 1], f32)
            nc.scalar.activation(out=lse_t, in_=sumexp_t, func=Act.Ln)
            picked_t = accp.tile([_BL, 1], f32)
            nc.vector.scalar_tensor_tensor(
                out=onehot_t,
                in0=scores_t,
                scalar=0.0,
                in1=onehot_t,
                op0=Alu.bypass,
                op1=Alu.mult,
                accum_out=picked_t,
            )
            # rowloss = rowmax + lse - picked
            nc.vector.tensor_tensor(
                out=lse_t, in0=lse_t, in1=rmax_t, op=Alu.add
            )
            nc.vector.tensor_tensor(
                out=out_t[0:_BL, 4:5], in0=lse_t, in1=picked_t, op=Alu.subtract
            )

            # ---- per-image spatial losses
            for b in range(_BL):
                locs_t = work.tile([_P, 4, _F], f32, tag="locs")
                nc.sync.dma_start(
                    out=locs_t, in_=locs_h[b].rearrange("c (p j) -> p c j", p=_P)
                )
                coords_t = work.tile([_P, 4, _F], f32, tag="coords")
                nc.sync.dma_start(
                    out=coords_t, in_=gt_h[b, 1:5].rearrange("c (p j) -> p c j", p=_P)
                )
                mask_t = work.tile([_P, _F], f32, tag="mask")
                nc.sync.dma_start(
                    out=mask_t, in_=gt_h[b, 0].rearrange("(p j) -> p j", p=_P)
                )
                obj_t = work.tile([_P, _F], f32, tag="obj")
                nc.sync.dma_start(
                    out=obj_t, in_=objects_h[b].rearrange("(p j) -> p j", p=_P)
                )

                # coord loss: sum(gt0*(l-c)^2) = sum(Square(d*m)), m in {0,1}
                d_t = work.tile([_P, 4, _F], f32, tag="d")
                nc.vector.tensor_tensor(
                    out=d_t, in0=locs_t, in1=coords_t, op=Alu.subtract
                )
                # mask broadcast over the channel dim (stride-0 AP)
                mask_b = bass.AP(
                    mask_t.tensor,
                    mask_t.offset,
                    [list(mask_t.ap[0]), [0, 4], list(mask_t.ap[1])],
                )
                dm_t = work.tile([_P, 4, _F], f32, tag="dm")
                nc.gpsimd.tensor_tensor(out=dm_t, in0=d_t, in1=mask_b, op=Alu.mult)
                nc.scalar.activation(
                    out=locs_t,  # scratch
                    in_=dm_t,
                    func=Act.Square,
                    accum_out=acc_sq[:, b : b + 1],
                )

                # BCE sums
                lnp_t = work.tile([_P, _F], f32, tag="lnp")
                nc.scalar.activation(out=lnp_t, in_=obj_t, func=Act.Ln)
                ln1m_t = work.tile([_P, _F], f32, tag="ln1m")
                nc.scalar.activation(
                    out=ln1m_t,
                    in_=obj_t,
                    func=Act.Ln,
                    scale=-1.0,
                    bias=1.0,
                    accum_out=acc_a[:, b : b + 1],
                )
                nc.vector.scalar_tensor_tensor(
                    out=lnp_t,
                    in0=lnp_t,
                    scalar=0.0,
                    in1=mask_t,
                    op0=Alu.bypass,
                    op1=Alu.mult,
                    accum_out=acc_c[:, b : b + 1],
                )
                nc.vector.scalar_tensor_tensor(
                    out=ln1m_t,
                    in0=ln1m_t,
                    scalar=0.0,
                    in1=mask_t,
                    op0=Alu.bypass,
                    op1=Alu.mult,
                    accum_out=acc_b[:, b : b + 1],
                )

            # final per-core reduction: [128, BL] -> [128, 1] per component
            nc.vector.reduce_sum(
                out=out_t[:, 0:1], in_=acc_sq, axis=mybir.AxisListType.X
            )
            nc.vector.reduce_sum(
                out=out_t[:, 1:2], in_=acc_a, axis=mybir.AxisListType.X
            )
            nc.vector.reduce_sum(
                out=out_t[:, 2:3], in_=acc_b, axis=mybir.AxisListType.X
            )
            nc.vector.reduce_sum(
                out=out_t[:, 3:4], in_=acc_c, axis=mybir.AxisListType.X
            )
            nc.sync.dma_start(out=out_h[:], in_=out_t)

    patched = _patch_multiwaits(mybir.module_to_json_bytes(nc.m))
    nc.to_json_bytes = lambda: patched
    return nc


def _shard_inputs(objects, scores, locs, label, gt):
    objects_r = np.asarray(objects, dtype=np.float32).reshape(_B, _HW)
    locs_r = np.asarray(locs, dtype=np.float32).reshape(_B, 4, _HW)
    gt_r = np.asarray(gt, dtype=np.float32).reshape(_B, 5, _HW)
    scores_r = np.asarray(scores, dtype=np.float32)
    label_i = np.asarray(label).astype(np.int64).reshape(_B)
    onehot = np.zeros((_B, _NCLS), dtype=np.float32)
    onehot[np.arange(_B), label_i] = 1.0
    in_maps = []
    for c in range(_NCORES):
        s = slice(c * _BL, (c + 1) * _BL)
        in_maps.append(
            {
                "objects": objects_r[s],
                "locs": locs_r[s],
                "gt": gt_r[s],
                "scores": scores_r[s],
                "onehot": onehot[s],
            }
        )
    return in_maps


def _run(in_maps, trace=False):
    global _PROG, LAST_RESULTS
    from concourse.bass_utils import run_bass_kernel_spmd

    if _PROG is None:
        _PROG = _build()
    res = run_bass_kernel_spmd(
        _PROG, in_maps, core_ids=list(range(_NCORES)), trace=trace
    )
    LAST_RESULTS = res
    return res.results


def kernel(objects, scores, locs, label, gt, obj_coor, no_obj_confi, img_class_weight):
    import os

    in_maps = _shard_inputs(objects, scores, locs, label, gt)
    trace = bool(int(os.environ.get("KERNEL_TRACE", "0")))
    outs = _run(in_maps, trace=trace)

    sq = a = b_ = c_ = cls = 0.0
    for om in outs:
        o = om["out"].astype(np.float64)
        sq += o[:, 0].sum()
        a += o[:, 1].sum()
        b_ += o[:, 2].sum()
        c_ += o[:, 3].sum()
        cls += o[0:_BL, 4].sum()

    obj_coor_loss = sq
    no_obj_confi_loss = -(a - b_)
    obj_confi_loss = -c_
    img_class_loss = cls / _B
    total = float(img_class_weight) * img_class_loss + (
        float(no_obj_confi) * no_obj_confi_loss
        + obj_confi_loss
        + float(obj_coor) * obj_coor_loss
    ) / _B
    return np.float32(total)
